# revision 9
# baseline (speedup 1.0000x reference)
"""Trainium2 Bass kernel for repeated sparse COO SpMM (GNN message passing).

y <- A @ y applied LAYERS times, A[row[e], col[e]] = weights[e].
N=100000 nodes, E=3200000 edges, B=16 features, 4 layers, 8 NeuronCores.

Strategy (1D partition by destination row + SBUF-resident y table):
  * Host: relabel nodes so each core owns a contiguous, degree-sorted,
    load-balanced range of destinations. Bucket each core's edges into
    per-destination slots so the on-chip segment-sum is a fixed-shape
    strided reduction.
  * y lives in SBUF as a table of 256B granules (2 rows of 64B per
    granule, 128B pad), 392 granules per partition. The SWDGE dma_gather
    instruction in SBUF-source mode computes addr = base + 256*idx, so a
    single int16 index reaches 32 partitions (256KB partition pitch =
    1024 granule units). Each slot gathers the WHOLE granule (128B, both
    rows); the per-sub-row weight mask (w on the edge's row, 0 on the
    other) selects the source during the multiply, so windows are
    partition-bases only (no parity fragmentation). Edges are
    host-assigned to one eligible window (balanced), and windows are
    clustered into groups with a uniform per-tile slot count inside each
    group so one strided 4D-AP DVE reduce per (tile, group) sums
    everything.
  * Gather calls rotate across 4 SWDGE queues so descriptor generation
    runs on all four Q7 core pairs concurrently; SBUF-source descriptors
    avoid the per-descriptor HBM latency that bounds DRAM-source
    gathers.
  * After each layer: AllGather the 8 compact per-core slices, then one
    strided DMA rebuilds the SBUF table from the gathered y.
"""

import numpy as np

# ---------------------------------------------------------------- problem dims
N_NODES = 100000
N_EDGES = 3200000
BATCH = 16
LAYERS = 4
NCORES = 8
P = 128

TROWS_PP = 784  # y rows per table partition (npad / 128)
GPP = TROWS_PP // 2  # 256B granules per table partition
PITCH_G = 1024  # granule units per partition pitch (256KB / 256B)
WREACH = 32  # table partitions reachable by one int16-indexed window
BASES_K = [0, 16, 32, 48, 64, 80, 96]  # window partition bases

CHUNK_COL_BUDGET = 150  # msg-buffer columns per chunk (x64B per partition)
NGROUPS = 3
REBALANCE_PASSES = 3
NUM_QUEUES = 4


class _Prep:
    """Host-side graph preprocessing, shared by kernel() and tests."""

    def __init__(self, x, weights, row, col, n_nodes, ncores, layers):
        n = n_nodes
        npc_real = n // ncores
        assert npc_real * ncores == n
        tiles = (npc_real + P - 1) // P
        npc = tiles * P
        npad = ncores * npc
        assert npad == P * TROWS_PP

        row = np.asarray(row).astype(np.int64)
        col = np.asarray(col).astype(np.int64)
        weights = np.asarray(weights, dtype=np.float32)
        deg = np.bincount(row, minlength=n)

        # ascending-degree order, snake-assigned to cores for load balance
        order = np.argsort(deg, kind="stable")
        blocks = order.reshape(npc_real, ncores).copy()
        blocks[1::2] = blocks[1::2, ::-1]
        perm = np.empty(n, dtype=np.int64)
        for c in range(ncores):
            perm[blocks[:, c]] = c * npc + np.arange(npc_real)

        new_row = perm[row]
        new_col = perm[col]

        wins = list(BASES_K)
        nw = len(wins)

        # --- balanced per-destination window assignment -----------------
        eorder = np.argsort(new_row, kind="stable")
        sr = new_row[eorder]
        sc = new_col[eorder]
        sw_weights = weights[eorder]
        change = np.flatnonzero(np.diff(sr)) + 1
        starts = np.concatenate(([0], change))
        counts = np.diff(np.concatenate((starts, [len(sr)])))
        dests = sr[starts]
        ndest = len(dests)
        maxdeg = int(counts.max()) if ndest else 0
        dest_ltile = (dests % npc) // P

        e_pt = sc // TROWS_PP  # table partition of the source row
        e_q = (sc % 2).astype(np.int64)  # sub-row within the granule
        elig = np.stack(
            [(e_pt >= k) & (e_pt < k + WREACH) for k in wins]
        )  # [nw, E]
        assert elig.any(axis=0).all()

        wassign = np.zeros(len(sr), dtype=np.int64)
        loads = np.zeros((ndest, nw), dtype=np.int64)
        BIG = 1 << 30
        for r in range(maxdeg):
            sel = counts > r
            epos = starts[sel] + r
            cost = np.where(elig[:, epos].T, loads[sel], BIG)
            pick = np.argmin(cost, axis=1)
            wassign[epos] = pick
            loads[sel, pick] += 1

        for _ in range(REBALANCE_PASSES):
            d_cur = np.zeros(tiles, dtype=np.int64)
            np.maximum.at(d_cur, dest_ltile, loads.max(axis=1))
            at_max = loads == d_cur[dest_ltile][:, None]
            moved = 0
            for di in np.flatnonzero(at_max.any(axis=1) & (counts > 1)):
                wmax = int(np.argmax(loads[di]))
                lo, hi = starts[di], starts[di] + counts[di]
                mine = np.arange(lo, hi)[wassign[lo:hi] == wmax]
                if len(mine) == 0:
                    continue
                el = elig[:, mine]
                best_w, best_e = -1, -1
                best_load = loads[di, wmax] - 1
                for w in range(nw):
                    if w == wmax:
                        continue
                    ok = np.flatnonzero(el[w])
                    if len(ok) and loads[di, w] < best_load:
                        best_w, best_e, best_load = w, mine[ok[0]], loads[di, w]
                if best_w >= 0:
                    wassign[best_e] = best_w
                    loads[di, wmax] -= 1
                    loads[di, best_w] += 1
                    moved += 1
            if moved == 0:
                break

        # --- per-(tile, window) slot maxima, window grouping ------------
        dtw = np.zeros((tiles, nw), dtype=np.int64)
        for w in range(nw):
            np.maximum.at(dtw[:, w], dest_ltile, loads[:, w])
        dtw = np.maximum(dtw, 1)

        ngroups = min(NGROUPS, nw)
        sums = dtw.sum(axis=0)
        order_w = np.argsort(sums)
        import itertools

        best = None
        for cuts in itertools.combinations(range(1, nw), ngroups - 1):
            groups = np.split(order_w, list(cuts))
            tot = sum(len(g) * dtw[:, g].max(axis=1).sum() for g in groups)
            if best is None or tot < best[0]:
                best = (tot, groups)
        groups = [list(map(int, g)) for g in best[1]]

        # D per (tile, group); per-window -> group id and position in group
        dtg = np.stack(
            [dtw[:, g].max(axis=1) for g in groups], axis=1
        )  # [tiles, ngroups]
        w2g = np.zeros(nw, dtype=np.int64)
        w2pos = np.zeros(nw, dtype=np.int64)
        for gi, g in enumerate(groups):
            for pi, w in enumerate(g):
                w2g[w] = gi
                w2pos[w] = pi
        gsize = np.array([len(g) for g in groups], dtype=np.int64)

        # --- chunks of tiles by column budget ---------------------------
        colw = (dtg * gsize[None, :]).sum(axis=1)  # msg columns per tile
        chunks = []  # (t0, t1)
        t0 = 0
        while t0 < tiles:
            t1 = t0
            acc = 0
            while t1 < tiles and (t1 == t0 or acc + colw[t1] <= CHUNK_COL_BUDGET):
                acc += colw[t1]
                t1 += 1
            chunks.append((t0, t1))
            t0 = t1
        nchunks = len(chunks)
        chunk_of_tile = np.zeros(tiles, dtype=np.int64)
        for ci, (a, b) in enumerate(chunks):
            chunk_of_tile[a:b] = ci

        # per-chunk per-group widths and offsets
        wcg = np.zeros((nchunks, ngroups), dtype=np.int64)  # sum of dtg in chunk
        for ci, (a, b) in enumerate(chunks):
            wcg[ci] = dtg[a:b].sum(axis=0)
        # column base of group section within a chunk buffer
        sec_base = np.zeros((nchunks, ngroups), dtype=np.int64)
        chunk_cols = np.zeros(nchunks, dtype=np.int64)
        for ci in range(nchunks):
            acc = 0
            for gi in range(ngroups):
                sec_base[ci, gi] = acc
                acc += gsize[gi] * wcg[ci, gi]
            chunk_cols[ci] = acc
        chunk_col_base = np.zeros(nchunks, dtype=np.int64)
        chunk_col_base[1:] = np.cumsum(chunk_cols)[:-1]
        total_cols = int(chunk_cols.sum())

        # tile offsets within (chunk, group): cumsum of dtg over chunk tiles
        offg = np.zeros((tiles, ngroups), dtype=np.int64)
        for ci, (a, b) in enumerate(chunks):
            offg[a:b] = np.cumsum(dtg[a:b], axis=0) - dtg[a:b]

        # --- per-edge slot index within its (dest, window) bucket -------
        grp_key = np.repeat(np.arange(ndest), counts) * nw + wassign
        gorder = np.argsort(grp_key, kind="stable")
        gs = grp_key[gorder]
        gchange = np.flatnonzero(np.diff(gs)) + 1
        gstarts = np.concatenate(([0], gchange))
        gcounts = np.diff(np.concatenate((gstarts, [len(gs)])))
        grun = np.repeat(np.arange(len(gstarts)), gcounts)
        j_sorted = np.arange(len(gs)) - gstarts[grun]
        j = np.empty(len(gs), dtype=np.int64)
        j[gorder] = j_sorted

        # --- per-edge column in the global w_s layout -------------------
        e_core = np.repeat(dests // npc, counts)
        e_ltile = np.repeat(dest_ltile, counts)
        e_p = np.repeat(dests % npc, counts) % P
        e_chunk = chunk_of_tile[e_ltile]
        e_g = w2g[wassign]
        e_wpos = w2pos[wassign]
        e_col = (
            chunk_col_base[e_chunk]
            + sec_base[e_chunk, e_g]
            + e_wpos * wcg[e_chunk, e_g]
            + offg[e_ltile, e_g]
            + j
        )

        w_all = np.zeros((ncores, P, 2 * total_cols), dtype=np.float32)
        w_all[e_core, e_p, 2 * e_col + e_q] = sw_weights

        # --- idx16 tables, one per (chunk, group, window-in-group) ------
        call_meta = []  # (chunk, group, wpos, window, entry_base, width)
        call_base = np.zeros((nchunks, nw), dtype=np.int64)  # by (chunk, w)
        acc2 = 0
        for ci in range(nchunks):
            for gi, g in enumerate(groups):
                for pi, w in enumerate(g):
                    call_base[ci, w] = acc2
                    call_meta.append(
                        (ci, gi, pi, w, int(acc2), int(wcg[ci, gi]))
                    )
                    acc2 += P * int(wcg[ci, gi])
        total_entries = int(acc2)

        chunk_entry_base = np.zeros(nchunks, dtype=np.int64)
        chunk_entries = np.zeros(nchunks, dtype=np.int64)
        for (ci, gi, pi, w, ebase, width) in call_meta:
            if chunk_entries[ci] == 0:
                chunk_entry_base[ci] = ebase
            chunk_entries[ci] += P * width

        wk = np.array(wins, dtype=np.int64)
        val = (e_pt - wk[wassign]) * PITCH_G + (sc % TROWS_PP) // 2
        assert (val >= 0).all() and (val < 32768).all()
        g_pos = (offg[e_ltile, e_g] + j) * P + e_p
        e_entry = call_base[e_chunk, wassign] + g_pos
        flat_idx = np.zeros((ncores, total_entries), dtype=np.int16)
        flat_idx[e_core, e_entry] = val.astype(np.int16)
        assert total_entries % 16 == 0
        wrapped = flat_idx.reshape(ncores, total_entries // 16, 16).transpose(
            0, 2, 1
        )
        idx16_all = np.ascontiguousarray(np.tile(wrapped, (1, 8, 1)))

        import ml_dtypes

        xpad = np.zeros((npad, BATCH), dtype=np.float32)
        xpad[perm, :] = np.asarray(x, dtype=np.float32)
        xpad = xpad.astype(ml_dtypes.bfloat16)

        self.n_nodes = n
        self.ncores = ncores
        self.layers = layers
        self.tiles = tiles
        self.npc = npc
        self.npad = npad
        self.nw = nw
        self.wins = wins
        self.ngroups = ngroups
        self.gsize = [int(v) for v in gsize]
        self.chunks = chunks
        self.wcg = wcg
        self.sec_base = sec_base
        self.chunk_cols = chunk_cols
        self.chunk_col_base = chunk_col_base
        self.dtg = dtg
        self.offg = offg
        self.call_meta = call_meta
        self.chunk_entry_base = chunk_entry_base
        self.chunk_entries = chunk_entries
        self.total_cols = total_cols
        self.total_entries = total_entries
        self.perm = perm
        self.w_all = w_all
        self.idx16_all = idx16_all
        self.xpad = xpad
        self.slots = int(total_cols) * P


_REG_CACHE = {}


def _dma_gather_sbuf_raw(
    g, out_ap, in_ap, idxs_ap, num_idxs, elem_size, byte_offset, queue_num
):
    """SBUF-source dma_gather: addr = in_base + byte_offset + 256*idx.

    Uses the ucode's sbuf-source branch with tokens_per_rank=1 so the
    per-index address math degenerates to a linear 256B-granule stride.
    """
    import concourse.mybir as mybir

    _in_ap = g.lower_ap(in_ap)
    _idxs_ap = g.lower_ap(idxs_ap)
    _out_ap = g.lower_ap(out_ap)
    key = (id(g), num_idxs)
    if key not in _REG_CACHE:
        _REG_CACHE[key] = g.to_reg(num_idxs)
    return g.add_instruction(
        mybir.InstDMAGatherAnt(
            name=g.bass.get_next_instruction_name(),
            ins=[_in_ap, _idxs_ap, g.lower_val_access(_REG_CACHE[key])],
            outs=[_out_ap],
            transpose=False,
            num_idxs=num_idxs,
            elem_size=elem_size,
            stride_bytes_256=1,
            gen_mode=0,
            single_packet=True,
            queue_num=queue_num,
            sbuf_tokens_per_rank=1,
            sbuf_free_dim_per_rank=256,
            sbuf_free_dim_pad_per_rank=0,
            sbuf_byte_offset=byte_offset,
        )
    )


def build_program(prep):
    import concourse.bass as bass
    import concourse.bacc as bacc
    import concourse.mybir as mybir
    import concourse.tile as tile

    ncores = prep.ncores
    npc = prep.npc
    npad = prep.npad
    layers = prep.layers
    ngroups = prep.ngroups

    nc = bacc.Bacc(
        None,
        num_devices=ncores,
        num_swdge_queues=NUM_QUEUES,
    )
    f32 = mybir.dt.float32
    bf16 = mybir.dt.bfloat16
    i16 = mybir.dt.int16
    xfull = nc.dram_tensor("xfull", [npad, BATCH], bf16, kind="ExternalInput")
    idx_d = nc.dram_tensor(
        "idx", [P, prep.total_entries // 16], i16, kind="ExternalInput"
    )
    w_d = nc.dram_tensor("w", [P, 2 * prep.total_cols], f32, kind="ExternalInput")
    yout = nc.dram_tensor("yout", [npc, BATCH], f32, kind="ExternalOutput")

    with tile.TileContext(nc) as tc:
        with (
            tc.tile_pool(name="res", bufs=1) as res_pool,
            tc.tile_pool(name="msgp", bufs=2) as msg_pool,
            tc.tile_pool(name="idxp", bufs=6) as idx_pool,
            tc.tile_pool(name="outp", bufs=1) as out_pool,
            tc.tile_pool(name="dram", bufs=1, space="DRAM") as dram_pool,
        ):
            w_s = res_pool.tile([P, 2 * prep.total_cols], f32, name="w_s")
            nc.sync.dma_start(out=w_s[:], in_=w_d[:])
            # prime the DVE dependency on the w_s load
            w_prime = res_pool.tile([P, 1], f32, name="w_prime")
            nc.vector.tensor_copy(out=w_prime[:], in_=w_s[:, 0:1])

            # y table: 392 granules x 256B per partition (2 rows + pad each)
            table = res_pool.tile([P, GPP * 128], bf16, name="table")

            slices = [
                dram_pool.tile([npc, BATCH], bf16, name=f"slice{i}")
                for i in range(max(layers - 1, 1))
            ]
            ags = [
                dram_pool.tile(
                    [npad, BATCH], bf16, addr_space="Shared", name=f"ag{i}"
                )
                for i in range(max(layers - 1, 1))
            ]

            def build_table(src):
                # src [npad, 16] bf16 -> table[p, g*128 : g*128+32] = row pair
                nc.sync.dma_start(
                    out=table[:].rearrange("p (g u) -> p g u", u=128)[
                        :, :, 0:32
                    ],
                    in_=src[:, :].rearrange(
                        "(p g two) f -> p g (two f)", p=P, two=2
                    ),
                )

            build_table(xfull)

            qn = 0
            for l in range(layers):
                dst = yout if l == layers - 1 else slices[l]
                ylayer = out_pool.tile(
                    [P, prep.tiles * BATCH], f32, name=f"ylayer{l}", tag=f"yl{l}"
                )
                for ci, (t0, t1) in enumerate(prep.chunks):
                    ccols = int(prep.chunk_cols[ci])
                    msg = msg_pool.tile(
                        [P, ccols * 2 * BATCH], bf16, name="msg", tag="msg"
                    )
                    # one idx load per chunk (entries are contiguous)
                    ch_e0 = int(prep.chunk_entry_base[ci])
                    ch_en = int(prep.chunk_entries[ci])
                    idxt = idx_pool.tile(
                        [P, ch_en // 16], i16, name="idxt", tag="idxt"
                    )
                    nc.sync.dma_start(
                        out=idxt[:],
                        in_=idx_d[:, ch_e0 // 16 : (ch_e0 + ch_en) // 16],
                    )
                    max_cols = 8  # 1024 idxs per call (SWDGE ring capacity)
                    for (ci2, gi, pi, w, ebase, width) in prep.call_meta:
                        if ci2 != ci:
                            continue
                        k_base = prep.wins[w]
                        sec0 = int(prep.sec_base[ci, gi]) + pi * width
                        for s0 in range(0, width, max_cols):
                            sw = min(max_cols, width - s0)
                            n_idx = P * sw
                            eb = ebase + P * s0 - ch_e0
                            cstart = (sec0 + s0) * 2 * BATCH
                            _dma_gather_sbuf_raw(
                                nc.gpsimd,
                                out_ap=msg[
                                    :, cstart : cstart + sw * 2 * BATCH
                                ].rearrange("p (c f) -> p c f", f=2 * BATCH),
                                # partition base k rides in sbuf_byte_offset:
                                # ucode adds (byte_offset//256)*256KB = k
                                # partition pitches to the (aligned) AP base.
                                in_ap=table[0:WREACH, :],
                                idxs_ap=idxt[:, eb // 16 : (eb + n_idx) // 16],
                                num_idxs=n_idx,
                                elem_size=2 * BATCH,
                                byte_offset=k_base * 256,
                                queue_num=qn,
                            )
                            qn = (qn + 1) % NUM_QUEUES
                    cb = int(prep.chunk_col_base[ci])
                    nc.vector.tensor_tensor(
                        out=msg[:].rearrange("p (d f) -> p d f", f=BATCH),
                        in0=msg[:].rearrange("p (d f) -> p d f", f=BATCH),
                        in1=w_s[:, 2 * cb : 2 * cb + 2 * ccols].to_broadcast(
                            [P, 2 * ccols, BATCH]
                        ),
                        op=mybir.AluOpType.mult,
                    )
                    for t in range(t0, t1):
                        for gi in range(ngroups):
                            d_tg = int(prep.dtg[t, gi])
                            gs = prep.gsize[gi]
                            o = (
                                int(prep.sec_base[ci, gi])
                                + int(prep.offg[t, gi])
                            ) * 2 * BATCH
                            base2 = msg[:, o : o + BATCH]
                            in_ap = bass.AP(
                                base2.tensor,
                                base2.offset,
                                [
                                    base2.ap[0],
                                    [1, BATCH],
                                    [int(prep.wcg[ci, gi]) * 2 * BATCH, gs],
                                    [BATCH, 2 * d_tg],
                                ],
                            )
                            if gi == 0:
                                nc.vector.tensor_reduce(
                                    out=ylayer[:, t * BATCH : (t + 1) * BATCH],
                                    in_=in_ap,
                                    axis=mybir.AxisListType.XY,
                                    op=mybir.AluOpType.add,
                                )
                            else:
                                tmp = out_pool.tile(
                                    [P, BATCH], f32, name="tmp", tag="tmp",
                                    bufs=4,
                                )
                                nc.vector.tensor_reduce(
                                    out=tmp[:],
                                    in_=in_ap,
                                    axis=mybir.AxisListType.XY,
                                    op=mybir.AluOpType.add,
                                )
                                nc.vector.tensor_add(
                                    out=ylayer[:, t * BATCH : (t + 1) * BATCH],
                                    in0=ylayer[:, t * BATCH : (t + 1) * BATCH],
                                    in1=tmp[:],
                                )
                if l == layers - 1:
                    nc.sync.dma_start(
                        out=dst[:, :].rearrange("(t p) f -> p t f", p=P),
                        in_=ylayer[:].rearrange("p (t f) -> p t f", f=BATCH),
                    )
                else:
                    nc.gpsimd.dma_start(
                        out=dst[:, :].rearrange("(t p) f -> p t f", p=P),
                        in_=ylayer[:].rearrange("p (t f) -> p t f", f=BATCH),
                    )
                if l < layers - 1:
                    nc.gpsimd.collective_compute(
                        "AllGather",
                        mybir.AluOpType.bypass,
                        replica_groups=[list(range(ncores))],
                        ins=[slices[l][:]],
                        outs=[ags[l][:]],
                    )
                    build_table(ags[l])
    nc.compile()
    return nc


def run(prep, trace=False):
    from concourse.bass_utils import run_bass_kernel_spmd

    nc = build_program(prep)
    in_maps = [
        {"xfull": prep.xpad, "idx": prep.idx16_all[k], "w": prep.w_all[k]}
        for k in range(prep.ncores)
    ]
    res = run_bass_kernel_spmd(
        nc, in_maps, core_ids=list(range(prep.ncores)), trace=trace
    )
    y_concat = np.concatenate(
        [res.results[k]["yout"] for k in range(prep.ncores)], axis=0
    )
    return y_concat[prep.perm], res


def kernel(x, weights, row, col):
    prep = _Prep(x, weights, row, col, N_NODES, NCORES, LAYERS)
    y, _ = run(prep, trace=False)
    return y


# revision 11
# speedup vs baseline: 1.0040x; 1.0040x over previous
"""Trainium2 Bass kernel for repeated sparse COO SpMM (GNN message passing).

y <- A @ y applied LAYERS times, A[row[e], col[e]] = weights[e].
N=100000 nodes, E=3200000 edges, B=16 features, 4 layers, 8 NeuronCores.

Strategy (1D partition by destination row + SBUF-resident y table):
  * Host: relabel nodes so each core owns a contiguous, degree-sorted,
    load-balanced range of destinations. Bucket each core's edges into
    per-destination slots so the on-chip segment-sum is a fixed-shape
    strided reduction.
  * y lives in SBUF as a table of 256B granules (2 rows of 64B per
    granule, 128B pad), 392 granules per partition. The SWDGE dma_gather
    instruction in SBUF-source mode computes addr = base + 256*idx, so a
    single int16 index reaches 32 partitions (256KB partition pitch =
    1024 granule units). Each slot gathers the WHOLE granule (128B, both
    rows); the per-sub-row weight mask (w on the edge's row, 0 on the
    other) selects the source during the multiply, so windows are
    partition-bases only (no parity fragmentation). Edges are
    host-assigned to one eligible window (balanced), and windows are
    clustered into groups with a uniform per-tile slot count inside each
    group so one strided 4D-AP DVE reduce per (tile, group) sums
    everything.
  * Gather calls rotate across 4 SWDGE queues so descriptor generation
    runs on all four Q7 core pairs concurrently; SBUF-source descriptors
    avoid the per-descriptor HBM latency that bounds DRAM-source
    gathers.
  * After each layer: AllGather the 8 compact per-core slices, then one
    strided DMA rebuilds the SBUF table from the gathered y.
"""

import numpy as np

# ---------------------------------------------------------------- problem dims
N_NODES = 100000
N_EDGES = 3200000
BATCH = 16
LAYERS = 4
NCORES = 8
P = 128

TROWS_PP = 784  # y rows per table partition (npad / 128)
GPP = TROWS_PP // 2  # 256B granules per table partition
PITCH_G = 1024  # granule units per partition pitch (256KB / 256B)
WREACH = 32  # table partitions reachable by one int16-indexed window
BASES_K = [0, 16, 32, 48, 64, 80, 96]  # window partition bases

CHUNK_COL_BUDGET = 150  # msg-buffer columns per chunk (x64B per partition)
NGROUPS = 3
REBALANCE_PASSES = 3
NUM_QUEUES = 4


class _Prep:
    """Host-side graph preprocessing, shared by kernel() and tests."""

    def __init__(self, x, weights, row, col, n_nodes, ncores, layers):
        n = n_nodes
        npc_real = n // ncores
        assert npc_real * ncores == n
        tiles = (npc_real + P - 1) // P
        npc = tiles * P
        npad = ncores * npc
        assert npad == P * TROWS_PP

        row = np.asarray(row).astype(np.int64)
        col = np.asarray(col).astype(np.int64)
        weights = np.asarray(weights, dtype=np.float32)
        deg = np.bincount(row, minlength=n)

        # ascending-degree order, snake-assigned to cores for load balance
        order = np.argsort(deg, kind="stable")
        blocks = order.reshape(npc_real, ncores).copy()
        blocks[1::2] = blocks[1::2, ::-1]
        perm = np.empty(n, dtype=np.int64)
        for c in range(ncores):
            perm[blocks[:, c]] = c * npc + np.arange(npc_real)

        new_row = perm[row]
        new_col = perm[col]

        wins = list(BASES_K)
        nw = len(wins)

        # --- balanced per-destination window assignment -----------------
        eorder = np.argsort(new_row, kind="stable")
        sr = new_row[eorder]
        sc = new_col[eorder]
        sw_weights = weights[eorder]
        change = np.flatnonzero(np.diff(sr)) + 1
        starts = np.concatenate(([0], change))
        counts = np.diff(np.concatenate((starts, [len(sr)])))
        dests = sr[starts]
        ndest = len(dests)
        maxdeg = int(counts.max()) if ndest else 0
        dest_ltile = (dests % npc) // P

        e_pt = sc // TROWS_PP  # table partition of the source row
        e_q = (sc % 2).astype(np.int64)  # sub-row within the granule
        elig = np.stack(
            [(e_pt >= k) & (e_pt < k + WREACH) for k in wins]
        )  # [nw, E]
        assert elig.any(axis=0).all()

        wassign = np.zeros(len(sr), dtype=np.int64)
        loads = np.zeros((ndest, nw), dtype=np.int64)
        BIG = 1 << 30
        for r in range(maxdeg):
            sel = counts > r
            epos = starts[sel] + r
            cost = np.where(elig[:, epos].T, loads[sel], BIG)
            pick = np.argmin(cost, axis=1)
            wassign[epos] = pick
            loads[sel, pick] += 1

        for _ in range(REBALANCE_PASSES):
            d_cur = np.zeros(tiles, dtype=np.int64)
            np.maximum.at(d_cur, dest_ltile, loads.max(axis=1))
            at_max = loads == d_cur[dest_ltile][:, None]
            moved = 0
            for di in np.flatnonzero(at_max.any(axis=1) & (counts > 1)):
                wmax = int(np.argmax(loads[di]))
                lo, hi = starts[di], starts[di] + counts[di]
                mine = np.arange(lo, hi)[wassign[lo:hi] == wmax]
                if len(mine) == 0:
                    continue
                el = elig[:, mine]
                best_w, best_e = -1, -1
                best_load = loads[di, wmax] - 1
                for w in range(nw):
                    if w == wmax:
                        continue
                    ok = np.flatnonzero(el[w])
                    if len(ok) and loads[di, w] < best_load:
                        best_w, best_e, best_load = w, mine[ok[0]], loads[di, w]
                if best_w >= 0:
                    wassign[best_e] = best_w
                    loads[di, wmax] -= 1
                    loads[di, best_w] += 1
                    moved += 1
            if moved == 0:
                break

        # --- per-(tile, window) slot maxima, window grouping ------------
        dtw = np.zeros((tiles, nw), dtype=np.int64)
        for w in range(nw):
            np.maximum.at(dtw[:, w], dest_ltile, loads[:, w])
        dtw = np.maximum(dtw, 1)

        ngroups = min(NGROUPS, nw)
        sums = dtw.sum(axis=0)
        order_w = np.argsort(sums)
        import itertools

        best = None
        for cuts in itertools.combinations(range(1, nw), ngroups - 1):
            groups = np.split(order_w, list(cuts))
            tot = sum(len(g) * dtw[:, g].max(axis=1).sum() for g in groups)
            if best is None or tot < best[0]:
                best = (tot, groups)
        groups = [list(map(int, g)) for g in best[1]]

        # D per (tile, group); per-window -> group id and position in group
        dtg = np.stack(
            [dtw[:, g].max(axis=1) for g in groups], axis=1
        )  # [tiles, ngroups]
        w2g = np.zeros(nw, dtype=np.int64)
        w2pos = np.zeros(nw, dtype=np.int64)
        for gi, g in enumerate(groups):
            for pi, w in enumerate(g):
                w2g[w] = gi
                w2pos[w] = pi
        gsize = np.array([len(g) for g in groups], dtype=np.int64)

        # --- chunks of tiles by column budget ---------------------------
        colw = (dtg * gsize[None, :]).sum(axis=1)  # msg columns per tile
        chunks = []  # (t0, t1)
        t0 = 0
        while t0 < tiles:
            t1 = t0
            acc = 0
            while t1 < tiles and (t1 == t0 or acc + colw[t1] <= CHUNK_COL_BUDGET):
                acc += colw[t1]
                t1 += 1
            chunks.append((t0, t1))
            t0 = t1
        nchunks = len(chunks)
        chunk_of_tile = np.zeros(tiles, dtype=np.int64)
        for ci, (a, b) in enumerate(chunks):
            chunk_of_tile[a:b] = ci

        # per-chunk per-group widths and offsets
        wcg = np.zeros((nchunks, ngroups), dtype=np.int64)  # sum of dtg in chunk
        for ci, (a, b) in enumerate(chunks):
            wcg[ci] = dtg[a:b].sum(axis=0)
        # column base of group section within a chunk buffer
        sec_base = np.zeros((nchunks, ngroups), dtype=np.int64)
        chunk_cols = np.zeros(nchunks, dtype=np.int64)
        for ci in range(nchunks):
            acc = 0
            for gi in range(ngroups):
                sec_base[ci, gi] = acc
                acc += gsize[gi] * wcg[ci, gi]
            chunk_cols[ci] = acc
        chunk_col_base = np.zeros(nchunks, dtype=np.int64)
        chunk_col_base[1:] = np.cumsum(chunk_cols)[:-1]
        total_cols = int(chunk_cols.sum())

        # tile offsets within (chunk, group): cumsum of dtg over chunk tiles
        offg = np.zeros((tiles, ngroups), dtype=np.int64)
        for ci, (a, b) in enumerate(chunks):
            offg[a:b] = np.cumsum(dtg[a:b], axis=0) - dtg[a:b]

        # --- per-edge slot index within its (dest, window) bucket -------
        grp_key = np.repeat(np.arange(ndest), counts) * nw + wassign
        gorder = np.argsort(grp_key, kind="stable")
        gs = grp_key[gorder]
        gchange = np.flatnonzero(np.diff(gs)) + 1
        gstarts = np.concatenate(([0], gchange))
        gcounts = np.diff(np.concatenate((gstarts, [len(gs)])))
        grun = np.repeat(np.arange(len(gstarts)), gcounts)
        j_sorted = np.arange(len(gs)) - gstarts[grun]
        j = np.empty(len(gs), dtype=np.int64)
        j[gorder] = j_sorted

        # --- per-edge column in the global w_s layout -------------------
        e_core = np.repeat(dests // npc, counts)
        e_ltile = np.repeat(dest_ltile, counts)
        e_p = np.repeat(dests % npc, counts) % P
        e_chunk = chunk_of_tile[e_ltile]
        e_g = w2g[wassign]
        e_wpos = w2pos[wassign]
        e_col = (
            chunk_col_base[e_chunk]
            + sec_base[e_chunk, e_g]
            + e_wpos * wcg[e_chunk, e_g]
            + offg[e_ltile, e_g]
            + j
        )

        w_all = np.zeros((ncores, P, 2 * total_cols), dtype=np.float32)
        w_all[e_core, e_p, 2 * e_col + e_q] = sw_weights

        # --- idx16 tables, one per (chunk, group, window-in-group) ------
        call_meta = []  # (chunk, group, wpos, window, entry_base, width)
        call_base = np.zeros((nchunks, nw), dtype=np.int64)  # by (chunk, w)
        acc2 = 0
        for ci in range(nchunks):
            for gi, g in enumerate(groups):
                for pi, w in enumerate(g):
                    call_base[ci, w] = acc2
                    call_meta.append(
                        (ci, gi, pi, w, int(acc2), int(wcg[ci, gi]))
                    )
                    acc2 += P * int(wcg[ci, gi])
        total_entries = int(acc2)

        chunk_entry_base = np.zeros(nchunks, dtype=np.int64)
        chunk_entries = np.zeros(nchunks, dtype=np.int64)
        for (ci, gi, pi, w, ebase, width) in call_meta:
            if chunk_entries[ci] == 0:
                chunk_entry_base[ci] = ebase
            chunk_entries[ci] += P * width

        wk = np.array(wins, dtype=np.int64)
        val = (e_pt - wk[wassign]) * PITCH_G + (sc % TROWS_PP) // 2
        assert (val >= 0).all() and (val < 32768).all()
        g_pos = (offg[e_ltile, e_g] + j) * P + e_p
        e_entry = call_base[e_chunk, wassign] + g_pos
        flat_idx = np.zeros((ncores, total_entries), dtype=np.int16)
        flat_idx[e_core, e_entry] = val.astype(np.int16)
        assert total_entries % 16 == 0
        wrapped = flat_idx.reshape(ncores, total_entries // 16, 16).transpose(
            0, 2, 1
        )
        idx16_all = np.ascontiguousarray(np.tile(wrapped, (1, 8, 1)))

        import ml_dtypes

        xpad = np.zeros((npad, BATCH), dtype=np.float32)
        xpad[perm, :] = np.asarray(x, dtype=np.float32)
        xpad = xpad.astype(ml_dtypes.bfloat16)

        self.n_nodes = n
        self.ncores = ncores
        self.layers = layers
        self.tiles = tiles
        self.npc = npc
        self.npad = npad
        self.nw = nw
        self.wins = wins
        self.ngroups = ngroups
        self.gsize = [int(v) for v in gsize]
        self.chunks = chunks
        self.wcg = wcg
        self.sec_base = sec_base
        self.chunk_cols = chunk_cols
        self.chunk_col_base = chunk_col_base
        self.dtg = dtg
        self.offg = offg
        self.call_meta = call_meta
        self.chunk_entry_base = chunk_entry_base
        self.chunk_entries = chunk_entries
        self.total_cols = total_cols
        self.total_entries = total_entries
        self.perm = perm
        self.w_all = w_all
        self.idx16_all = idx16_all
        self.xpad = xpad
        self.slots = int(total_cols) * P


_REG_CACHE = {}


def _dma_gather_sbuf_raw(
    g, out_ap, in_ap, idxs_ap, num_idxs, elem_size, byte_offset, queue_num
):
    """SBUF-source dma_gather: addr = in_base + byte_offset + 256*idx.

    Uses the ucode's sbuf-source branch with tokens_per_rank=1 so the
    per-index address math degenerates to a linear 256B-granule stride.
    """
    import concourse.mybir as mybir

    _in_ap = g.lower_ap(in_ap)
    _idxs_ap = g.lower_ap(idxs_ap)
    _out_ap = g.lower_ap(out_ap)
    key = (id(g), num_idxs)
    if key not in _REG_CACHE:
        _REG_CACHE[key] = g.to_reg(num_idxs)
    return g.add_instruction(
        mybir.InstDMAGatherAnt(
            name=g.bass.get_next_instruction_name(),
            ins=[_in_ap, _idxs_ap, g.lower_val_access(_REG_CACHE[key])],
            outs=[_out_ap],
            transpose=False,
            num_idxs=num_idxs,
            elem_size=elem_size,
            stride_bytes_256=1,
            gen_mode=0,
            single_packet=True,
            queue_num=queue_num,
            sbuf_tokens_per_rank=1,
            sbuf_free_dim_per_rank=256,
            sbuf_free_dim_pad_per_rank=0,
            sbuf_byte_offset=byte_offset,
        )
    )


def build_program(prep):
    import concourse.bass as bass
    import concourse.bacc as bacc
    import concourse.mybir as mybir
    import concourse.tile as tile

    ncores = prep.ncores
    npc = prep.npc
    npad = prep.npad
    layers = prep.layers
    ngroups = prep.ngroups

    nc = bacc.Bacc(
        None,
        num_devices=ncores,
        num_swdge_queues=NUM_QUEUES,
    )
    f32 = mybir.dt.float32
    bf16 = mybir.dt.bfloat16
    i16 = mybir.dt.int16
    xfull = nc.dram_tensor("xfull", [npad, BATCH], bf16, kind="ExternalInput")
    idx_d = nc.dram_tensor(
        "idx", [P, prep.total_entries // 16], i16, kind="ExternalInput"
    )
    w_d = nc.dram_tensor("w", [P, 2 * prep.total_cols], f32, kind="ExternalInput")
    yout = nc.dram_tensor("yout", [npc, BATCH], f32, kind="ExternalOutput")

    with tile.TileContext(nc) as tc:
        with (
            tc.tile_pool(name="res", bufs=1) as res_pool,
            tc.tile_pool(name="msgp", bufs=2) as msg_pool,
            tc.tile_pool(name="idxp", bufs=6) as idx_pool,
            tc.tile_pool(name="outp", bufs=1) as out_pool,
            tc.tile_pool(name="dram", bufs=1, space="DRAM") as dram_pool,
        ):
            w_s = res_pool.tile([P, 2 * prep.total_cols], f32, name="w_s")
            nc.sync.dma_start(out=w_s[:], in_=w_d[:])
            # prime the DVE dependency on the w_s load
            w_prime = res_pool.tile([P, 1], f32, name="w_prime")
            nc.vector.tensor_copy(out=w_prime[:], in_=w_s[:, 0:1])

            # y table: 392 granules x 256B per partition (2 rows + pad each)
            table = res_pool.tile([P, GPP * 128], bf16, name="table")

            slices = [
                dram_pool.tile([npc, BATCH], bf16, name=f"slice{i}")
                for i in range(max(layers - 1, 1))
            ]
            stags = [
                dram_pool.tile([npc, BATCH], f32, name=f"stag{i}")
                for i in range(max(layers - 1, 1))
            ]
            ags = [
                dram_pool.tile(
                    [npad, BATCH], bf16, addr_space="Shared", name=f"ag{i}"
                )
                for i in range(max(layers - 1, 1))
            ]

            def build_table(src):
                # src [npad, 16] bf16 -> table[p, g*128 : g*128+32] = row pair
                nc.sync.dma_start(
                    out=table[:].rearrange("p (g u) -> p g u", u=128)[
                        :, :, 0:32
                    ],
                    in_=src[:, :].rearrange(
                        "(p g two) f -> p g (two f)", p=P, two=2
                    ),
                )

            build_table(xfull)

            qn = 0
            for l in range(layers):
                dst = yout if l == layers - 1 else slices[l]
                ylayer = out_pool.tile(
                    [P, prep.tiles * BATCH], f32, name=f"ylayer{l}", tag=f"yl{l}"
                )
                for ci, (t0, t1) in enumerate(prep.chunks):
                    ccols = int(prep.chunk_cols[ci])
                    msg = msg_pool.tile(
                        [P, ccols * 2 * BATCH], bf16, name="msg", tag="msg"
                    )
                    # one idx load per chunk (entries are contiguous)
                    ch_e0 = int(prep.chunk_entry_base[ci])
                    ch_en = int(prep.chunk_entries[ci])
                    idxt = idx_pool.tile(
                        [P, ch_en // 16], i16, name="idxt", tag="idxt"
                    )
                    nc.sync.dma_start(
                        out=idxt[:],
                        in_=idx_d[:, ch_e0 // 16 : (ch_e0 + ch_en) // 16],
                    )
                    max_cols = 8  # 1024 idxs per call (SWDGE ring capacity)
                    for (ci2, gi, pi, w, ebase, width) in prep.call_meta:
                        if ci2 != ci:
                            continue
                        k_base = prep.wins[w]
                        sec0 = int(prep.sec_base[ci, gi]) + pi * width
                        for s0 in range(0, width, max_cols):
                            sw = min(max_cols, width - s0)
                            n_idx = P * sw
                            eb = ebase + P * s0 - ch_e0
                            cstart = (sec0 + s0) * 2 * BATCH
                            _dma_gather_sbuf_raw(
                                nc.gpsimd,
                                out_ap=msg[
                                    :, cstart : cstart + sw * 2 * BATCH
                                ].rearrange("p (c f) -> p c f", f=2 * BATCH),
                                # partition base k rides in sbuf_byte_offset:
                                # ucode adds (byte_offset//256)*256KB = k
                                # partition pitches to the (aligned) AP base.
                                in_ap=table[0:WREACH, :],
                                idxs_ap=idxt[:, eb // 16 : (eb + n_idx) // 16],
                                num_idxs=n_idx,
                                elem_size=2 * BATCH,
                                byte_offset=k_base * 256,
                                queue_num=qn,
                            )
                            qn = (qn + 1) % NUM_QUEUES
                    cb = int(prep.chunk_col_base[ci])
                    nc.vector.tensor_tensor(
                        out=msg[:].rearrange("p (d f) -> p d f", f=BATCH),
                        in0=msg[:].rearrange("p (d f) -> p d f", f=BATCH),
                        in1=w_s[:, 2 * cb : 2 * cb + 2 * ccols].to_broadcast(
                            [P, 2 * ccols, BATCH]
                        ),
                        op=mybir.AluOpType.mult,
                    )
                    for t in range(t0, t1):
                        for gi in range(ngroups):
                            d_tg = int(prep.dtg[t, gi])
                            gs = prep.gsize[gi]
                            o = (
                                int(prep.sec_base[ci, gi])
                                + int(prep.offg[t, gi])
                            ) * 2 * BATCH
                            base2 = msg[:, o : o + BATCH]
                            in_ap = bass.AP(
                                base2.tensor,
                                base2.offset,
                                [
                                    base2.ap[0],
                                    [1, BATCH],
                                    [int(prep.wcg[ci, gi]) * 2 * BATCH, gs],
                                    [BATCH, 2 * d_tg],
                                ],
                            )
                            if gi == 0:
                                nc.vector.tensor_reduce(
                                    out=ylayer[:, t * BATCH : (t + 1) * BATCH],
                                    in_=in_ap,
                                    axis=mybir.AxisListType.XY,
                                    op=mybir.AluOpType.add,
                                )
                            else:
                                tmp = out_pool.tile(
                                    [P, BATCH], f32, name="tmp", tag="tmp",
                                    bufs=4,
                                )
                                nc.vector.tensor_reduce(
                                    out=tmp[:],
                                    in_=in_ap,
                                    axis=mybir.AxisListType.XY,
                                    op=mybir.AluOpType.add,
                                )
                                nc.vector.tensor_add(
                                    out=ylayer[:, t * BATCH : (t + 1) * BATCH],
                                    in0=ylayer[:, t * BATCH : (t + 1) * BATCH],
                                    in1=tmp[:],
                                )
                    # stream this chunk's rows out on the Sync engine while
                    # later chunks still gather (keeps the Pool engine free)
                    cdst = yout if l == layers - 1 else stags[l]
                    nc.sync.dma_start(
                        out=cdst[t0 * P : t1 * P, :].rearrange(
                            "(t p) f -> p t f", p=P
                        ),
                        in_=ylayer[:, t0 * BATCH : t1 * BATCH].rearrange(
                            "p (t f) -> p t f", f=BATCH
                        ),
                    )
                if l < layers - 1:
                    # cheap contiguous cast f32 -> bf16 (few big descriptors)
                    nc.gpsimd.dma_start(out=slices[l][:, :], in_=stags[l][:, :])
                if l < layers - 1:
                    nc.gpsimd.collective_compute(
                        "AllGather",
                        mybir.AluOpType.bypass,
                        replica_groups=[list(range(ncores))],
                        ins=[slices[l][:]],
                        outs=[ags[l][:]],
                    )
                    build_table(ags[l])
    nc.compile()
    return nc


def run(prep, trace=False):
    from concourse.bass_utils import run_bass_kernel_spmd

    nc = build_program(prep)
    in_maps = [
        {"xfull": prep.xpad, "idx": prep.idx16_all[k], "w": prep.w_all[k]}
        for k in range(prep.ncores)
    ]
    res = run_bass_kernel_spmd(
        nc, in_maps, core_ids=list(range(prep.ncores)), trace=trace
    )
    y_concat = np.concatenate(
        [res.results[k]["yout"] for k in range(prep.ncores)], axis=0
    )
    return y_concat[prep.perm], res


def kernel(x, weights, row, col):
    prep = _Prep(x, weights, row, col, N_NODES, NCORES, LAYERS)
    y, _ = run(prep, trace=False)
    return y
